# revision 7
# baseline (speedup 1.0000x reference)
"""CTC loss kernel for Trainium2, 8-core SPMD, data-parallel over batch.

- Shard B=64 examples as 8 per core.
- Phase A (per 128-timestep tile): DMA logits, logsumexp over classes (no
  max subtraction; inputs are N(0,1)), gather label-class logits with a
  one-hot fp32 matmul (exact), subtract lse, and transpose into a
  resident SBUF "Q" buffer of per-step log-probs laid out for the DP
  (label position on partitions).
- Phase B: two-lane CTC forward DP in log space. Label-dimension shifts
  run on the PE as permutation matmuls (exact data movement); empty slots
  are filled with -1e30 by a rank-1 inject matmul. logaddexp(a,b) is
  computed as max(a,b) + ln(1 + exp(-|a-b|)) with the exp/ln batched on
  the scalar engine (both live in one activation table).
- Freezing past each example's input length (last 256 steps only): cross
  terms are killed with an additive -1e30 column mask, per-step log-probs
  with a multiplicative 0/1 mask, so frozen columns update as
  alpha' = alpha exactly.
- Host: builds one-hot/skip/freeze tables, reads the two lattice values
  per example, logaddexp, zero_infinity, /target_len, batch mean.

State layout (free dim, 40 cols = 5 groups x 8 examples, col = g*8+e):
  g0: blank lane s in [0,128)   g1: blank lane s in [128,256)
  g2: label lane s in [0,128)   g3: label lane s in [128,256)
  g4: blank s=256 (row 0 only; rows 1..127 stay -1e30)
"""

import sys

sys.path.insert(0, "/opt/trn_rl_repo")

import numpy as np

B, T, C, S = 64, 2048, 512, 256
NCORES = 8
EXPC = B // NCORES
TBLK = 256
NEG = -1.0e30

_cache = {}


def _build_program(T_, TBLK_, tail_start):
    import concourse.bacc as bacc
    import concourse.bass as bass
    import concourse.tile as tile
    from concourse import mybir

    dt = mybir.dt
    AF = mybir.ActivationFunctionType
    OP = mybir.AluOpType
    AP = bass.AP

    NBLK = T_ // TBLK_
    TAIL = T_ - tail_start

    nc = bacc.Bacc("TRN2", target_bir_lowering=False, debug=False,
                   num_devices=NCORES)

    preds = nc.dram_tensor("preds", [EXPC, T_, C], dt.float32,
                           kind="ExternalInput")
    oh = nc.dram_tensor("oh", [EXPC, 4, 128, 257], dt.float32,
                        kind="ExternalInput")
    sks_d = nc.dram_tensor("sks", [128, 16], dt.float32, kind="ExternalInput")
    g01_d = nc.dram_tensor("gtab01", [TAIL + 1, 8], dt.float32,
                           kind="ExternalInput")
    gm_d = nc.dram_tensor("gtabm", [TAIL + 1, 8], dt.float32,
                          kind="ExternalInput")
    mats_d = nc.dram_tensor("mats", [3, 128, 128], dt.float32,
                            kind="ExternalInput")
    negs_d = nc.dram_tensor("negs", [1, 24], dt.float32, kind="ExternalInput")
    e0_d = nc.dram_tensor("e0row", [1, 128], dt.float32, kind="ExternalInput")
    ones_d = nc.dram_tensor("onesrow", [1, 128], dt.float32,
                            kind="ExternalInput")
    out_alpha = nc.dram_tensor("out_alpha", [128, 40], dt.float32,
                               kind="ExternalOutput")

    def dap(t, off, dims):
        return AP(t, off, dims)

    with tile.TileContext(nc) as tc:
        with (
            tc.tile_pool(name="state", bufs=1) as st,
            tc.tile_pool(name="qpool", bufs=1) as qp,
            tc.tile_pool(name="ldpool", bufs=3) as ldp,
            tc.tile_pool(name="work", bufs=2) as wk,
            tc.tile_pool(name="psB", bufs=2, space="PSUM") as psB,
            tc.tile_pool(name="psA", bufs=2, space="PSUM") as psA,
            tc.tile_pool(name="psZ", bufs=1, space="PSUM") as psZ,
            tc.tile_pool(name="psQ", bufs=2, space="PSUM") as psQ,
        ):
            f32 = dt.float32
            alpha = st.tile([128, 40], f32)
            lmL = st.tile([128, 16], f32)
            abar = st.tile([128, 32], f32)
            sks = st.tile([128, 16], f32)
            g01src = st.tile([1, (TAIL + 1) * 8], f32)
            gmsrc = st.tile([1, (TAIL + 1) * 8], f32)
            gb01 = [st.tile([128, 8], f32, tag=f"gb01_{i}", name=f"gb01_{i}")
                    for i in range(2)]
            gbm = [st.tile([128, 8], f32, tag=f"gbm_{i}", name=f"gbm_{i}")
                   for i in range(2)]
            mats = st.tile([128, 3 * 128], f32)
            negs = st.tile([1, 24], f32)
            e0row = st.tile([1, 128], f32)
            onesrow = st.tile([1, 128], f32)
            qbuf = [qp.tile([128, TBLK_ * 40], f32, tag=f"qb{i}",
                            name=f"qb{i}") for i in range(2)]

            IM = mats[:, 0:128]
            S1 = mats[:, 128:256]
            E127 = mats[:, 256:384]

            nc.sync.dma_start(sks[:], sks_d.ap())
            nc.sync.dma_start(
                g01src[:],
                dap(g01_d, 0, [[(TAIL + 1) * 8, 1], [1, (TAIL + 1) * 8]]))
            nc.sync.dma_start(
                gmsrc[:],
                dap(gm_d, 0, [[(TAIL + 1) * 8, 1], [1, (TAIL + 1) * 8]]))
            for c in range(3):
                nc.sync.dma_start(
                    mats[:, c * 128:(c + 1) * 128],
                    dap(mats_d, c * 128 * 128, [[128, 128], [1, 128]]))
            nc.sync.dma_start(negs[:], negs_d.ap())
            nc.sync.dma_start(e0row[:], e0_d.ap())
            nc.sync.dma_start(onesrow[:], ones_d.ap())

            def phase_a(blk):
                Q = qbuf[blk % 2]
                for tloc in range(TBLK_ // 128):
                    tt = blk * (TBLK_ // 128) + tloc
                    t0 = tt * 128
                    for e in range(EXPC):
                        lg = ldp.tile([128, 512], f32, tag="lg", name="lg")
                        nc.sync.dma_start(
                            lg[:],
                            dap(preds, e * T_ * C + t0 * C,
                                [[C, 128], [1, C]]))
                        ohS = ldp.tile([128, 4 * 257], f32, tag="ohS",
                                       name="ohS")
                        nc.sync.dma_start(
                            ohS[:],
                            dap(oh, e * 4 * 128 * 257,
                                [[257, 128], [128 * 257, 4], [1, 257]]))
                        exps = wk.tile([128, 512], f32, tag="exps",
                                       name="exps")
                        esum = wk.tile([128, 1], f32, tag="esum", name="esum")
                        nc.scalar.activation(exps[:], lg[:], AF.Exp,
                                             accum_out=esum[:, 0:1])
                        lnsum = wk.tile([128, 1], f32, tag="lnsum",
                                        name="lnsum")
                        nc.scalar.activation(lnsum[:], esum[:], AF.Ln)
                        nlse = wk.tile([128, 1], f32, tag="nlse", name="nlse")
                        nc.vector.tensor_scalar(nlse[:], lnsum[:], -1.0, None,
                                                OP.mult)
                        ltS = wk.tile([128, 512], f32, tag="ltS", name="ltS")
                        for c in range(4):
                            ltP = psA.tile([128, 128], f32, tag="ltP",
                                           name="ltP")
                            nc.tensor.matmul(ltP[:],
                                             lg[:, c * 128:(c + 1) * 128],
                                             IM, is_transpose=True,
                                             start=True, stop=True,
                                             skip_group_check=True)
                            if c < 2:
                                nc.scalar.activation(
                                    ltS[:, c * 128:(c + 1) * 128], ltP[:],
                                    AF.Copy)
                            else:
                                nc.vector.tensor_copy(
                                    ltS[:, c * 128:(c + 1) * 128], ltP[:])
                        z = psZ.tile([128, 257], f32, tag="z", name="z")
                        for c in range(4):
                            nc.tensor.matmul(
                                z[:], ltS[:, c * 128:(c + 1) * 128],
                                ohS[:, c * 257:(c + 1) * 257],
                                start=(c == 0), stop=(c == 3))
                        qS = wk.tile([128, 257], f32, tag="qS", name="qS")
                        nc.vector.tensor_scalar(qS[:], z[:], nlse[:, 0:1],
                                                None, OP.add)
                        qTP = psQ.tile([128, 512], f32, tag="qTP", name="qTP")
                        nc.tensor.matmul(qTP[:, 0:128], qS[:, 1:129], IM,
                                         is_transpose=True, start=True,
                                         stop=True, skip_group_check=True)
                        nc.tensor.matmul(qTP[:, 128:256], qS[:, 129:257], IM,
                                         is_transpose=True, start=True,
                                         stop=True, skip_group_check=True)
                        nc.tensor.matmul(qTP[:, 256:384], qS[:, 0:128], IM,
                                         is_transpose=True, start=True,
                                         stop=True, skip_group_check=True)
                        qTbS = wk.tile([1, 128], f32, tag="qTbS", name="qTbS")
                        nc.vector.tensor_copy(qTbS[:], qTP[0:1, 256:384])
                        nc.tensor.matmul(qTP[:, 384:512], onesrow[0:1, :],
                                         qTbS[0:1, :], start=True, stop=True,
                                         skip_group_check=True)
                        base = tloc * 128 * 40
                        in_l = AP(qTP[:].tensor, qTP[:].offset,
                                  [qTP[:].ap[0], [128, 2], [1, 128]])
                        out_l = AP(Q[:].tensor, Q[:].offset + base + 16 + e,
                                   [Q[:].ap[0], [8, 2], [40, 128]])
                        nc.scalar.activation(out_l, in_l, AF.Copy)
                        in_b = AP(qTP[:].tensor, qTP[:].offset + 384,
                                  [qTP[:].ap[0], [0, 2], [1, 128]])
                        out_b = AP(Q[:].tensor, Q[:].offset + base + 0 + e,
                                   [Q[:].ap[0], [8, 2], [40, 128]])
                        nc.scalar.activation(out_b, in_b, AF.Copy)
                        in_b2 = AP(qTP[:].tensor, qTP[:].offset + 384,
                                   [qTP[:].ap[0], [1, 128]])
                        out_b2 = AP(Q[:].tensor, Q[:].offset + base + 32 + e,
                                    [Q[:].ap[0], [40, 128]])
                        nc.scalar.activation(out_b2, in_b2, AF.Copy)

            def qslice(t, lo, hi):
                Q = qbuf[(t // TBLK_) % 2]
                off = (t % TBLK_) * 40 + lo
                return AP(Q[:].tensor, Q[:].offset + off,
                          [Q[:].ap[0], [1, hi - lo]])

            def qrow(t, lo, hi):
                a = qslice(t, lo, hi)
                return AP(a.tensor, a.offset, [[a.ap[0][0], 1], [1, hi - lo]])

            def bview(t8, ngrp):
                a = t8[:]
                return AP(a.tensor, a.offset, [a.ap[0], [0, ngrp], [1, 8]])

            def pbc(dst, srctile, idx):
                nc.gpsimd.partition_broadcast(
                    dst[:],
                    AP(srctile[:].tensor, srctile[:].offset + idx * 8,
                       [[srctile[:].ap[0][0], 1], [1, 8]]))

            # ---- init ----
            phase_a(0)
            nc.vector.memset(alpha[:], NEG)
            nc.vector.tensor_copy(alpha[0:1, 0:8], qrow(0, 0, 8))
            nc.vector.tensor_copy(alpha[0:1, 16:24], qrow(0, 16, 24))
            nc.vector.tensor_add(lmL[:], sks[:], alpha[:, 16:32])

            for t in range(1, T_):
                blk = t // TBLK_
                if t % TBLK_ == 1 and blk + 1 < NBLK:
                    phase_a(blk + 1)
                tail = t >= tail_start
                if t == tail_start:
                    pbc(gb01[t % 2], g01src, t - tail_start)
                    pbc(gbm[t % 2], gmsrc, t - tail_start)
                    nc.vector.tensor_add(abar[:], alpha[:, 0:32],
                                         bview(gbm[t % 2], 4))
                    nc.vector.tensor_add(lmL[:], sks[:], abar[:, 16:32])
                src = abar if tail else alpha

                P = psB.tile([128, 40], f32, tag="P", name="P")
                nc.tensor.matmul(P[:, 0:16], S1, src[:, 16:32],
                                 start=True, stop=False,
                                 skip_group_check=True)
                nc.tensor.matmul(P[:, 8:16], E127, src[:, 16:24],
                                 start=False, stop=False,
                                 skip_group_check=True)
                nc.tensor.matmul(P[:, 0:16], e0row[0:1, :], negs[0:1, 0:16],
                                 start=False, stop=False,
                                 skip_group_check=True)
                nc.tensor.matmul(P[:, 16:24], E127, src[:, 24:32],
                                 start=True, stop=False,
                                 skip_group_check=True)
                nc.tensor.matmul(P[:, 24:40], S1, lmL[:, 0:16],
                                 start=True, stop=False,
                                 skip_group_check=True)
                nc.tensor.matmul(P[:, 32:40], E127, lmL[:, 0:8],
                                 start=False, stop=False,
                                 skip_group_check=True)
                nc.tensor.matmul(P[:, 24:32], e0row[0:1, :], negs[0:1, 16:24],
                                 start=False, stop=True,
                                 skip_group_check=True)
                # P cols: 0:16 sh (l[s-1]) for b-lane; 16:24 sh256 (row 0);
                #         24:40 skip-shift for l-lane

                D12 = wk.tile([128, 32], f32, tag="D12", name="D12")
                D34 = wk.tile([128, 24], f32, tag="D34", name="D34")
                TMP = wk.tile([128, 32], f32, tag="TMP", name="TMP")
                m1 = wk.tile([128, 16], f32, tag="m1", name="m1")
                m2 = wk.tile([128, 16], f32, tag="m2", name="m2")
                u = wk.tile([128, 16], f32, tag="u", name="u")
                m3 = wk.tile([128, 16], f32, tag="m3", name="m3")
                m4 = wk.tile([1, 8], f32, tag="m4", name="m4")
                d1 = wk.tile([128, 16], f32, tag="d1", name="d1")
                d2 = wk.tile([128, 16], f32, tag="d2", name="d2")
                d3 = wk.tile([128, 16], f32, tag="d3", name="d3")
                d4 = wk.tile([1, 8], f32, tag="d4", name="d4")
                E12 = wk.tile([128, 32], f32, tag="E12", name="E12")
                L12 = wk.tile([128, 32], f32, tag="L12", name="L12")
                E34 = wk.tile([128, 24], f32, tag="E34", name="E34")
                L34 = wk.tile([128, 24], f32, tag="L34", name="L34")

                bsrc = src  # masked in tail, alpha otherwise
                # b-lane: la2(alpha_b, sh)
                nc.vector.tensor_max(m1[:], alpha[:, 0:16], P[:, 0:16])
                nc.vector.tensor_sub(d1[:], alpha[:, 0:16], P[:, 0:16])
                nc.vector.scalar_tensor_tensor(D12[:, 0:16], d1[:], -1.0,
                                               d1[:], OP.mult, OP.max)
                # l-lane stage1: la2(alpha_l, b-masked)
                nc.vector.tensor_max(m2[:], alpha[:, 16:32], bsrc[:, 0:16])
                nc.vector.tensor_sub(d2[:], alpha[:, 16:32], bsrc[:, 0:16])
                nc.vector.scalar_tensor_tensor(D12[:, 16:32], d2[:], -1.0,
                                               d2[:], OP.mult, OP.max)
                nc.scalar.activation(E12[:], D12[:], AF.Exp, scale=-1.0)
                nc.scalar.activation(L12[:], E12[:], AF.Ln, bias=1.0)
                nc.vector.tensor_add(TMP[:, 0:16], m1[:], L12[:, 0:16])
                nc.vector.tensor_add(u[:], m2[:], L12[:, 16:32])
                # l-lane stage2: la2(u, skipshift)
                nc.vector.tensor_max(m3[:], u[:], P[:, 24:40])
                nc.vector.tensor_sub(d3[:], u[:], P[:, 24:40])
                nc.vector.scalar_tensor_tensor(D34[:, 0:16], d3[:], -1.0,
                                               d3[:], OP.mult, OP.max)
                # b256: la2(alpha_b256, sh256)
                nc.vector.memset(D34[:, 16:24], 0.0)
                nc.vector.tensor_max(m4[:], alpha[0:1, 32:40], P[0:1, 16:24])
                nc.vector.tensor_sub(d4[:], alpha[0:1, 32:40], P[0:1, 16:24])
                nc.vector.scalar_tensor_tensor(D34[0:1, 16:24], d4[:], -1.0,
                                               d4[:], OP.mult, OP.max)
                nc.scalar.activation(E34[:], D34[:], AF.Exp, scale=-1.0)
                nc.scalar.activation(L34[:], E34[:], AF.Ln, bias=1.0)
                nc.vector.tensor_add(TMP[:, 16:32], m3[:], L34[:, 0:16])
                v4 = wk.tile([1, 8], f32, tag="v4", name="v4")
                nc.vector.tensor_add(v4[:], m4[:], L34[0:1, 16:24])

                if tail:
                    tp = wk.tile([128, 40], f32, tag="tp", name="tp")
                    nc.vector.tensor_mul(tp[:], qslice(t, 0, 40),
                                         bview(gb01[t % 2], 5))
                    nc.vector.tensor_add(alpha[:, 0:32], TMP[:, 0:32],
                                         tp[:, 0:32])
                    nc.vector.tensor_add(alpha[0:1, 32:40], v4[:],
                                         tp[0:1, 32:40])
                else:
                    nc.vector.tensor_add(alpha[:, 0:32], TMP[:, 0:32],
                                         qslice(t, 0, 32))
                    nc.vector.tensor_add(alpha[0:1, 32:40], v4[:],
                                         qrow(t, 32, 40))

                last = t == T_ - 1
                if tail and not last:
                    pbc(gb01[(t + 1) % 2], g01src, t + 1 - tail_start)
                    pbc(gbm[(t + 1) % 2], gmsrc, t + 1 - tail_start)
                    nc.vector.tensor_add(abar[:], alpha[:, 0:32],
                                         bview(gbm[(t + 1) % 2], 4))
                    nc.vector.tensor_add(lmL[:], sks[:], abar[:, 16:32])
                elif not last:
                    nc.vector.tensor_add(lmL[:], sks[:], alpha[:, 16:32])

            nc.sync.dma_start(out_alpha.ap(), alpha[:])

    nc.compile()
    return nc


def _host_tables(targets_k, pred_lens_k, tail_start, T_):
    TAIL = T_ - tail_start
    ohm = np.zeros((EXPC, 4, 128, 257), np.float32)
    for e in range(EXPC):
        ohm[e, 0, 0, 0] = 1.0
        y = targets_k[e]
        for s in range(S):
            cls = int(y[s])
            ohm[e, cls // 128, cls % 128, 1 + s] = 1.0
    sks = np.full((128, 16), NEG, np.float32)
    for e in range(EXPC):
        y = targets_k[e]
        for s in range(S - 1):
            if y[s + 1] != y[s]:
                sks[s % 128, (s // 128) * 8 + e] = 0.0
    g01 = np.zeros((TAIL + 1, 8), np.float32)
    gm = np.zeros((TAIL + 1, 8), np.float32)
    for i in range(TAIL + 1):
        t = tail_start + i
        act = (t < pred_lens_k)
        g01[i] = act.astype(np.float32)
        gm[i] = np.where(act, 0.0, NEG).astype(np.float32)
    mats = np.zeros((3, 128, 128), np.float32)
    mats[0] = np.eye(128, dtype=np.float32)
    mats[1] = np.eye(128, k=1, dtype=np.float32)
    mats[2, 127, 0] = 1.0
    negs = np.zeros((1, 24), np.float32)
    negs[0, 0:8] = NEG
    negs[0, 16:24] = NEG
    e0row = np.zeros((1, 128), np.float32)
    e0row[0, 0] = 1.0
    return {
        "oh": ohm, "sks": sks, "gtab01": g01, "gtabm": gm, "mats": mats,
        "negs": negs, "e0row": e0row,
        "onesrow": np.ones((1, 128), np.float32),
    }


def _postprocess(results, targets, pred_lens, tgt_lens):
    losses = np.zeros(B, np.float64)
    for k in range(NCORES):
        a = np.asarray(results[k]["out_alpha"], np.float64)
        for e in range(EXPC):
            b = k * EXPC + e
            tl = int(tgt_lens[b])
            if tl == 256:
                v_end = a[0, 32 + e]
            elif tl >= 128:
                v_end = a[tl - 128, 8 + e]
            else:
                v_end = a[tl, 0 + e]
            s1 = tl - 1
            if s1 < 0:
                v_end1 = NEG
            elif s1 >= 128:
                v_end1 = a[s1 - 128, 24 + e]
            else:
                v_end1 = a[s1, 16 + e]
            loss = -np.logaddexp(v_end, v_end1)
            if not (loss < 1e29):
                loss = 0.0
            losses[b] = loss / max(tl, 1)
    return np.float32(losses.mean())


def kernel(predictions, targets, predictions_lengths, target_lengths):
    return run_full(predictions, targets, predictions_lengths,
                    target_lengths)[0]


def run_full(predictions, targets, predictions_lengths, target_lengths,
             trace=False):
    from concourse.bass_utils import run_bass_kernel_spmd

    T_ = predictions.shape[1]
    tail_start = T_ - TBLK
    key = (T_, TBLK, tail_start)
    if key not in _cache:
        _cache[key] = _build_program(T_, TBLK, tail_start)
    nc = _cache[key]

    predictions = np.ascontiguousarray(predictions, dtype=np.float32)
    targets = np.asarray(targets)
    pred_lens = np.asarray(predictions_lengths)
    tgt_lens = np.asarray(target_lengths)

    in_maps = []
    for k in range(NCORES):
        sl = slice(k * EXPC, (k + 1) * EXPC)
        tabs = _host_tables(targets[sl], pred_lens[sl], tail_start, T_)
        m = {"preds": np.ascontiguousarray(predictions[sl])}
        m.update(tabs)
        in_maps.append(m)

    bkr = run_bass_kernel_spmd(nc, in_maps, list(range(NCORES)),
                               trace=trace)
    return _postprocess(bkr.results, targets, pred_lens, tgt_lens), bkr


# revision 8
# speedup vs baseline: 1.0950x; 1.0950x over previous
"""CTC loss kernel for Trainium2, 8-core SPMD, data-parallel over batch.

- Shard B=64 examples as 8 per core.
- Phase A (per 128-timestep tile): DMA logits, logsumexp over classes (no
  max subtraction; inputs are N(0,1)), gather label-class logits with a
  one-hot fp32 matmul (exact), subtract lse, and transpose into a
  resident SBUF "Q" buffer of per-step log-probs laid out for the DP
  (label position on partitions).
- Phase B: two-lane CTC forward DP in log space. Label-dimension shifts
  run on the PE as permutation matmuls (exact data movement); empty slots
  are filled with -1e30 by a rank-1 inject matmul. logaddexp(a,b) is
  computed as max(a,b) + ln(1 + exp(-|a-b|)) with the exp/ln batched on
  the scalar engine (both live in one activation table).
- Freezing past each example's input length (last 256 steps only): cross
  terms are killed with an additive -1e30 column mask, per-step log-probs
  with a multiplicative 0/1 mask, so frozen columns update as
  alpha' = alpha exactly.
- Host: builds one-hot/skip/freeze tables, reads the two lattice values
  per example, logaddexp, zero_infinity, /target_len, batch mean.

State layout (free dim, 40 cols = 5 groups x 8 examples, col = g*8+e):
  g0: blank lane s in [0,128)   g1: blank lane s in [128,256)
  g2: label lane s in [0,128)   g3: label lane s in [128,256)
  g4: blank s=256 (row 0 only; rows 1..127 stay -1e30)
"""

import sys

sys.path.insert(0, "/opt/trn_rl_repo")

import numpy as np

B, T, C, S = 64, 2048, 512, 256
NCORES = 8
EXPC = B // NCORES
TBLK = 256
NEG = -1.0e30

_cache = {}


def _build_program(T_, TBLK_, tail_start):
    import concourse.bacc as bacc
    import concourse.bass as bass
    import concourse.tile as tile
    from concourse import mybir

    dt = mybir.dt
    AF = mybir.ActivationFunctionType
    OP = mybir.AluOpType
    AP = bass.AP

    NBLK = T_ // TBLK_
    TAIL = T_ - tail_start

    nc = bacc.Bacc("TRN2", target_bir_lowering=False, debug=False,
                   num_devices=NCORES)

    preds = nc.dram_tensor("preds", [EXPC, T_, C], dt.float32,
                           kind="ExternalInput")
    oh = nc.dram_tensor("oh", [EXPC, 4, 128, 257], dt.float32,
                        kind="ExternalInput")
    sks_d = nc.dram_tensor("sks", [128, 16], dt.float32, kind="ExternalInput")
    g01_d = nc.dram_tensor("gtab01", [TAIL + 1, 8], dt.float32,
                           kind="ExternalInput")
    gm_d = nc.dram_tensor("gtabm", [TAIL + 1, 8], dt.float32,
                          kind="ExternalInput")
    mats_d = nc.dram_tensor("mats", [3, 128, 128], dt.float32,
                            kind="ExternalInput")
    negs_d = nc.dram_tensor("negs", [1, 24], dt.float32, kind="ExternalInput")
    e0_d = nc.dram_tensor("e0row", [1, 128], dt.float32, kind="ExternalInput")
    ones_d = nc.dram_tensor("onesrow", [1, 128], dt.float32,
                            kind="ExternalInput")
    out_alpha = nc.dram_tensor("out_alpha", [128, 40], dt.float32,
                               kind="ExternalOutput")

    def dap(t, off, dims):
        return AP(t, off, dims)

    with tile.TileContext(nc) as tc:
        with (
            tc.tile_pool(name="state", bufs=1) as st,
            tc.tile_pool(name="qpool", bufs=1) as qp,
            tc.tile_pool(name="ldpool", bufs=3) as ldp,
            tc.tile_pool(name="work", bufs=2) as wk,
            tc.tile_pool(name="psB", bufs=2, space="PSUM") as psB,
            tc.tile_pool(name="psA", bufs=2, space="PSUM") as psA,
            tc.tile_pool(name="psZ", bufs=1, space="PSUM") as psZ,
            tc.tile_pool(name="psQ", bufs=2, space="PSUM") as psQ,
        ):
            f32 = dt.float32
            alpha = st.tile([128, 40], f32)
            lmL = st.tile([128, 16], f32)
            abar = st.tile([128, 32], f32)
            sks = st.tile([128, 16], f32)
            g01src = st.tile([1, (TAIL + 1) * 8], f32)
            gmsrc = st.tile([1, (TAIL + 1) * 8], f32)
            gb01 = [st.tile([128, 8], f32, tag=f"gb01_{i}", name=f"gb01_{i}")
                    for i in range(2)]
            gbm = [st.tile([128, 8], f32, tag=f"gbm_{i}", name=f"gbm_{i}")
                   for i in range(2)]
            mats = st.tile([128, 3 * 128], f32)
            negs = st.tile([1, 24], f32)
            e0row = st.tile([1, 128], f32)
            onesrow = st.tile([1, 128], f32)
            qbuf = [qp.tile([128, TBLK_ * 40], f32, tag=f"qb{i}",
                            name=f"qb{i}") for i in range(2)]

            IM = mats[:, 0:128]
            S1 = mats[:, 128:256]
            E127 = mats[:, 256:384]

            nc.sync.dma_start(sks[:], sks_d.ap())
            nc.sync.dma_start(
                g01src[:],
                dap(g01_d, 0, [[(TAIL + 1) * 8, 1], [1, (TAIL + 1) * 8]]))
            nc.sync.dma_start(
                gmsrc[:],
                dap(gm_d, 0, [[(TAIL + 1) * 8, 1], [1, (TAIL + 1) * 8]]))
            for c in range(3):
                nc.sync.dma_start(
                    mats[:, c * 128:(c + 1) * 128],
                    dap(mats_d, c * 128 * 128, [[128, 128], [1, 128]]))
            nc.sync.dma_start(negs[:], negs_d.ap())
            nc.sync.dma_start(e0row[:], e0_d.ap())
            nc.sync.dma_start(onesrow[:], ones_d.ap())

            def phase_a(blk):
                Q = qbuf[blk % 2]
                for tloc in range(TBLK_ // 128):
                    tt = blk * (TBLK_ // 128) + tloc
                    t0 = tt * 128
                    for e in range(EXPC):
                        lg = ldp.tile([128, 512], f32, tag="lg", name="lg")
                        nc.sync.dma_start(
                            lg[:],
                            dap(preds, e * T_ * C + t0 * C,
                                [[C, 128], [1, C]]))
                        ohS = ldp.tile([128, 4 * 257], f32, tag="ohS",
                                       name="ohS")
                        nc.sync.dma_start(
                            ohS[:],
                            dap(oh, e * 4 * 128 * 257,
                                [[257, 128], [128 * 257, 4], [1, 257]]))
                        exps = wk.tile([128, 512], f32, tag="exps",
                                       name="exps")
                        esum = wk.tile([128, 1], f32, tag="esum", name="esum")
                        nc.scalar.activation(exps[:], lg[:], AF.Exp,
                                             accum_out=esum[:, 0:1])
                        lnsum = wk.tile([128, 1], f32, tag="lnsum",
                                        name="lnsum")
                        nc.scalar.activation(lnsum[:], esum[:], AF.Ln)
                        nlse = wk.tile([128, 1], f32, tag="nlse", name="nlse")
                        nc.vector.tensor_scalar(nlse[:], lnsum[:], -1.0, None,
                                                OP.mult)
                        ltS = wk.tile([128, 512], f32, tag="ltS", name="ltS")
                        for c in range(4):
                            ltP = psA.tile([128, 128], f32, tag="ltP",
                                           name="ltP")
                            nc.tensor.matmul(ltP[:],
                                             lg[:, c * 128:(c + 1) * 128],
                                             IM, is_transpose=True,
                                             start=True, stop=True,
                                             skip_group_check=True)
                            if c < 2:
                                nc.scalar.activation(
                                    ltS[:, c * 128:(c + 1) * 128], ltP[:],
                                    AF.Copy)
                            else:
                                nc.vector.tensor_copy(
                                    ltS[:, c * 128:(c + 1) * 128], ltP[:])
                        z = psZ.tile([128, 257], f32, tag="z", name="z")
                        for c in range(4):
                            nc.tensor.matmul(
                                z[:], ltS[:, c * 128:(c + 1) * 128],
                                ohS[:, c * 257:(c + 1) * 257],
                                start=(c == 0), stop=(c == 3))
                        qS = wk.tile([128, 257], f32, tag="qS", name="qS")
                        nc.vector.tensor_scalar(qS[:], z[:], nlse[:, 0:1],
                                                None, OP.add)
                        qTP = psQ.tile([128, 512], f32, tag="qTP", name="qTP")
                        nc.tensor.matmul(qTP[:, 0:128], qS[:, 1:129], IM,
                                         is_transpose=True, start=True,
                                         stop=True, skip_group_check=True)
                        nc.tensor.matmul(qTP[:, 128:256], qS[:, 129:257], IM,
                                         is_transpose=True, start=True,
                                         stop=True, skip_group_check=True)
                        nc.tensor.matmul(qTP[:, 256:384], qS[:, 0:128], IM,
                                         is_transpose=True, start=True,
                                         stop=True, skip_group_check=True)
                        qTbS = wk.tile([1, 128], f32, tag="qTbS", name="qTbS")
                        nc.vector.tensor_copy(qTbS[:], qTP[0:1, 256:384])
                        nc.tensor.matmul(qTP[:, 384:512], onesrow[0:1, :],
                                         qTbS[0:1, :], start=True, stop=True,
                                         skip_group_check=True)
                        base = tloc * 128 * 40
                        in_l = AP(qTP[:].tensor, qTP[:].offset,
                                  [qTP[:].ap[0], [128, 2], [1, 128]])
                        out_l = AP(Q[:].tensor, Q[:].offset + base + 16 + e,
                                   [Q[:].ap[0], [8, 2], [40, 128]])
                        nc.scalar.activation(out_l, in_l, AF.Copy)
                        in_b = AP(qTP[:].tensor, qTP[:].offset + 384,
                                  [qTP[:].ap[0], [0, 2], [1, 128]])
                        out_b = AP(Q[:].tensor, Q[:].offset + base + 0 + e,
                                   [Q[:].ap[0], [8, 2], [40, 128]])
                        nc.scalar.activation(out_b, in_b, AF.Copy)
                        in_b2 = AP(qTP[:].tensor, qTP[:].offset + 384,
                                   [qTP[:].ap[0], [1, 128]])
                        out_b2 = AP(Q[:].tensor, Q[:].offset + base + 32 + e,
                                    [Q[:].ap[0], [40, 128]])
                        nc.scalar.activation(out_b2, in_b2, AF.Copy)

            def qslice(t, lo, hi):
                Q = qbuf[(t // TBLK_) % 2]
                off = (t % TBLK_) * 40 + lo
                return AP(Q[:].tensor, Q[:].offset + off,
                          [Q[:].ap[0], [1, hi - lo]])

            def qrow(t, lo, hi):
                a = qslice(t, lo, hi)
                return AP(a.tensor, a.offset, [[a.ap[0][0], 1], [1, hi - lo]])

            def bview(t8, ngrp):
                a = t8[:]
                return AP(a.tensor, a.offset, [a.ap[0], [0, ngrp], [1, 8]])

            def pbc(dst, srctile, idx):
                nc.gpsimd.partition_broadcast(
                    dst[:],
                    AP(srctile[:].tensor, srctile[:].offset + idx * 8,
                       [[srctile[:].ap[0][0], 1], [1, 8]]))

            # ---- init ----
            phase_a(0)
            nc.vector.memset(alpha[:], NEG)
            nc.vector.tensor_copy(alpha[0:1, 0:8], qrow(0, 0, 8))
            nc.vector.tensor_copy(alpha[0:1, 16:24], qrow(0, 16, 24))
            nc.vector.tensor_add(lmL[:], sks[:], alpha[:, 16:32])

            for t in range(1, T_):
                blk = t // TBLK_
                if t % TBLK_ == 1 and blk + 1 < NBLK:
                    phase_a(blk + 1)
                tail = t >= tail_start
                if t == tail_start:
                    pbc(gb01[t % 2], g01src, t - tail_start)
                    pbc(gbm[t % 2], gmsrc, t - tail_start)
                    nc.vector.tensor_add(abar[:], alpha[:, 0:32],
                                         bview(gbm[t % 2], 4))
                    nc.vector.tensor_add(lmL[:], sks[:], abar[:, 16:32])
                src = abar if tail else alpha

                P = psB.tile([128, 40], f32, tag="P", name="P")
                nc.tensor.matmul(P[:, 0:16], S1, src[:, 16:32],
                                 start=True, stop=False,
                                 skip_group_check=True)
                nc.tensor.matmul(P[:, 8:16], E127, src[:, 16:24],
                                 start=False, stop=False,
                                 skip_group_check=True)
                nc.tensor.matmul(P[:, 0:16], e0row[0:1, :], negs[0:1, 0:16],
                                 start=False, stop=False,
                                 skip_group_check=True)
                nc.tensor.matmul(P[:, 16:24], E127, src[:, 24:32],
                                 start=True, stop=False,
                                 skip_group_check=True)
                nc.tensor.matmul(P[:, 24:40], S1, lmL[:, 0:16],
                                 start=True, stop=False,
                                 skip_group_check=True)
                nc.tensor.matmul(P[:, 32:40], E127, lmL[:, 0:8],
                                 start=False, stop=False,
                                 skip_group_check=True)
                nc.tensor.matmul(P[:, 24:32], e0row[0:1, :], negs[0:1, 16:24],
                                 start=False, stop=True,
                                 skip_group_check=True)
                # P cols: 0:16 sh (l[s-1]) for b-lane; 16:24 sh256 (row 0);
                #         24:40 skip-shift for l-lane

                D12 = wk.tile([128, 32], f32, tag="D12", name="D12")
                D34 = wk.tile([128, 24], f32, tag="D34", name="D34")
                TMP = wk.tile([128, 32], f32, tag="TMP", name="TMP")
                m1 = wk.tile([128, 16], f32, tag="m1", name="m1")
                m2 = wk.tile([128, 16], f32, tag="m2", name="m2")
                u = wk.tile([128, 16], f32, tag="u", name="u")
                m3 = wk.tile([128, 16], f32, tag="m3", name="m3")
                m4 = wk.tile([1, 8], f32, tag="m4", name="m4")
                d1 = wk.tile([128, 16], f32, tag="d1", name="d1")
                d2 = wk.tile([128, 16], f32, tag="d2", name="d2")
                d3 = wk.tile([128, 16], f32, tag="d3", name="d3")
                d4 = wk.tile([1, 8], f32, tag="d4", name="d4")
                E12 = wk.tile([128, 32], f32, tag="E12", name="E12")
                L12 = wk.tile([128, 32], f32, tag="L12", name="L12")
                E34 = wk.tile([128, 24], f32, tag="E34", name="E34")
                L34 = wk.tile([128, 24], f32, tag="L34", name="L34")

                bsrc = src  # masked in tail, alpha otherwise
                # b-lane: la2(alpha_b, sh)
                nc.vector.tensor_max(m1[:], alpha[:, 0:16], P[:, 0:16])
                nc.vector.tensor_sub(d1[:], alpha[:, 0:16], P[:, 0:16])
                nc.vector.scalar_tensor_tensor(D12[:, 0:16], d1[:], -1.0,
                                               d1[:], OP.mult, OP.max)
                # l-lane stage1: la2(alpha_l, b-masked)
                nc.vector.tensor_max(m2[:], alpha[:, 16:32], bsrc[:, 0:16])
                nc.vector.tensor_sub(d2[:], alpha[:, 16:32], bsrc[:, 0:16])
                nc.vector.scalar_tensor_tensor(D12[:, 16:32], d2[:], -1.0,
                                               d2[:], OP.mult, OP.max)
                nc.scalar.activation(E12[:], D12[:], AF.Exp, scale=-1.0)
                nc.scalar.activation(L12[:], E12[:], AF.Ln, bias=1.0)
                nc.vector.tensor_add(TMP[:, 0:16], m1[:], L12[:, 0:16])
                nc.vector.tensor_add(u[:], m2[:], L12[:, 16:32])
                # l-lane stage2: la2(u, skipshift)
                nc.vector.tensor_max(m3[:], u[:], P[:, 24:40])
                nc.vector.tensor_sub(d3[:], u[:], P[:, 24:40])
                nc.vector.scalar_tensor_tensor(D34[:, 0:16], d3[:], -1.0,
                                               d3[:], OP.mult, OP.max)
                # b256: la2(alpha_b256, sh256)
                nc.vector.memset(D34[:, 16:24], 0.0)
                nc.vector.tensor_max(m4[:], alpha[0:1, 32:40], P[0:1, 16:24])
                nc.vector.tensor_sub(d4[:], alpha[0:1, 32:40], P[0:1, 16:24])
                nc.vector.scalar_tensor_tensor(D34[0:1, 16:24], d4[:], -1.0,
                                               d4[:], OP.mult, OP.max)
                nc.scalar.activation(E34[:], D34[:], AF.Exp, scale=-1.0)
                nc.scalar.activation(L34[:], E34[:], AF.Ln, bias=1.0)
                nc.vector.tensor_add(TMP[:, 16:32], m3[:], L34[:, 0:16])
                v4 = wk.tile([1, 8], f32, tag="v4", name="v4")
                nc.vector.tensor_add(v4[:], m4[:], L34[0:1, 16:24])

                if tail:
                    tp = wk.tile([128, 40], f32, tag="tp", name="tp")
                    nc.vector.tensor_mul(tp[:], qslice(t, 0, 40),
                                         bview(gb01[t % 2], 5))
                    nc.vector.tensor_add(alpha[:, 0:32], TMP[:, 0:32],
                                         tp[:, 0:32])
                    nc.vector.tensor_add(alpha[0:1, 32:40], v4[:],
                                         tp[0:1, 32:40])
                else:
                    nc.vector.tensor_add(alpha[:, 0:32], TMP[:, 0:32],
                                         qslice(t, 0, 32))
                    nc.vector.tensor_add(alpha[0:1, 32:40], v4[:],
                                         qrow(t, 32, 40))

                last = t == T_ - 1
                if tail and not last:
                    pbc(gb01[(t + 1) % 2], g01src, t + 1 - tail_start)
                    pbc(gbm[(t + 1) % 2], gmsrc, t + 1 - tail_start)
                    nc.vector.tensor_add(abar[:], alpha[:, 0:32],
                                         bview(gbm[(t + 1) % 2], 4))
                    nc.vector.tensor_add(lmL[:], sks[:], abar[:, 16:32])
                elif not last:
                    nc.vector.tensor_add(lmL[:], sks[:], alpha[:, 16:32])

            nc.sync.dma_start(out_alpha.ap(), alpha[:])

    nc.compile()
    return nc


def _host_tables(targets_k, pred_lens_k, tail_start, T_):
    TAIL = T_ - tail_start
    y = np.asarray(targets_k)
    ohm = np.zeros((EXPC, 4, 128, 257), np.float32)
    ohm[:, 0, 0, 0] = 1.0
    ee = np.repeat(np.arange(EXPC), S)
    yr = y.ravel()
    jj = np.tile(np.arange(1, S + 1), EXPC)
    ohm[ee, yr // 128, yr % 128, jj] = 1.0
    skmask = np.zeros((S, EXPC), bool)
    skmask[0:S - 1] = (y[:, 1:] != y[:, :-1]).T
    sks = np.where(skmask, 0.0, NEG).astype(np.float32)
    sks = sks.reshape(2, 128, EXPC).transpose(1, 0, 2).reshape(128, 16)
    t_arr = tail_start + np.arange(TAIL + 1)
    act = t_arr[:, None] < np.asarray(pred_lens_k)[None, :]
    g01 = act.astype(np.float32)
    gm = np.where(act, 0.0, NEG).astype(np.float32)
    mats = np.zeros((3, 128, 128), np.float32)
    mats[0] = np.eye(128, dtype=np.float32)
    mats[1] = np.eye(128, k=1, dtype=np.float32)
    mats[2, 127, 0] = 1.0
    negs = np.zeros((1, 24), np.float32)
    negs[0, 0:8] = NEG
    negs[0, 16:24] = NEG
    e0row = np.zeros((1, 128), np.float32)
    e0row[0, 0] = 1.0
    return {
        "oh": ohm, "sks": sks, "gtab01": g01, "gtabm": gm, "mats": mats,
        "negs": negs, "e0row": e0row,
        "onesrow": np.ones((1, 128), np.float32),
    }


def _postprocess(results, targets, pred_lens, tgt_lens):
    losses = np.zeros(B, np.float64)
    for k in range(NCORES):
        a = np.asarray(results[k]["out_alpha"], np.float64)
        for e in range(EXPC):
            b = k * EXPC + e
            tl = int(tgt_lens[b])
            if tl == 256:
                v_end = a[0, 32 + e]
            elif tl >= 128:
                v_end = a[tl - 128, 8 + e]
            else:
                v_end = a[tl, 0 + e]
            s1 = tl - 1
            if s1 < 0:
                v_end1 = NEG
            elif s1 >= 128:
                v_end1 = a[s1 - 128, 24 + e]
            else:
                v_end1 = a[s1, 16 + e]
            loss = -np.logaddexp(v_end, v_end1)
            if not (loss < 1e29):
                loss = 0.0
            losses[b] = loss / max(tl, 1)
    return np.float32(losses.mean())


def kernel(predictions, targets, predictions_lengths, target_lengths):
    return run_full(predictions, targets, predictions_lengths,
                    target_lengths)[0]


def run_full(predictions, targets, predictions_lengths, target_lengths,
             trace=False):
    from concourse.bass_utils import run_bass_kernel_spmd

    T_ = predictions.shape[1]
    tail_start = T_ - TBLK
    key = (T_, TBLK, tail_start)
    if key not in _cache:
        _cache[key] = _build_program(T_, TBLK, tail_start)
    nc = _cache[key]

    predictions = np.ascontiguousarray(predictions, dtype=np.float32)
    targets = np.asarray(targets)
    pred_lens = np.asarray(predictions_lengths)
    tgt_lens = np.asarray(target_lengths)

    in_maps = []
    for k in range(NCORES):
        sl = slice(k * EXPC, (k + 1) * EXPC)
        tabs = _host_tables(targets[sl], pred_lens[sl], tail_start, T_)
        m = {"preds": np.ascontiguousarray(predictions[sl])}
        m.update(tabs)
        in_maps.append(m)

    bkr = run_bass_kernel_spmd(nc, in_maps, list(range(NCORES)),
                               trace=trace)
    return _postprocess(bkr.results, targets, pred_lens, tgt_lens), bkr
